# revision 4
# baseline (speedup 1.0000x reference)
"""AffinityPropagate prediction kernel for Trainium2 (8 NeuronCores).

Data-parallel over batch B=8: each core owns one image [480, 640].

Layout per core: 120 partitions x 4 image rows, rows padded to 642 cols.
State g is fp16 double-buffered with 2 halo row slots per partition,
g_k = f_k * 2^-k: the per-iteration 1/2 rescale is folded into the
weights (w' = w/2, exact), so each iteration is pure fp16:

  9 tap products in 3 batched DVE muls (overlapping-window 4-dim APs,
  taps grouped by dc; fp16 tensor_tensor runs in DVE 2x mode at any
  byte offset) + a 4-instruction batched add tree. Halo rows refresh
  via 2 SBUF->SBUF DMAs overlapped with the interior half of the last
  add. Weight prep (normalize, center, *0.5) runs during the affinity
  load using 4x-mode tensor_scalar abs/copies. Output is one 2x-mode
  tensor_scalar upscale by 2^times then DMA.
"""

import numpy as np
from contextlib import ExitStack

import bass_rust
import concourse.bacc as bacc
import concourse.mybir as mybir
import concourse.tile as tile
from concourse import bass_utils
from concourse.bass_interp import get_hw_module

B, CH, H, W = 8, 8, 480, 640
P = 120             # partitions (each holds R rows)
R = H // P          # 4
SLOTS = R + 2       # + top/bottom halo row slots
WPAD = W + 2        # [0, img cols at 1..640, 0]
PX = R * W          # 2560 compact px per partition

F32 = mybir.dt.float32
FP16 = mybir.dt.float16
AF = mybir.ActivationFunctionType
OP = mybir.AluOpType

# w9/pr slab order: 0..2 = dc=-1 (dr=-1,0,1), 3..5 = dc=+1, 6..8 = dc=0
# (dr=-1, center, dr=+1).  Affinity channel c (reference tap order with
# center removed) -> slab:
CH2SLAB = {0: 0, 3: 1, 5: 2, 2: 3, 4: 4, 7: 5, 1: 6, 6: 8}
CEN_SLAB = 7


def _build(times: int):
    nc = bacc.Bacc("TRN2", debug=False, dynamic_dma_scratch_size=2048)
    aff_d = nc.dram_tensor("affinity", [CH, H * W], F32, kind="ExternalInput")
    feat_d = nc.dram_tensor("feature", [H, W], F32, kind="ExternalInput")
    out_d = nc.dram_tensor("out", [H, W], F32, kind="ExternalOutput")

    with tile.TileContext(nc) as tc, ExitStack() as ctx:
        pool = ctx.enter_context(tc.tile_pool(name="main", bufs=1))

        w9 = pool.tile([P, 9 * PX], FP16)
        pr = pool.tile([P, 9 * PX], FP16)
        gb = [pool.tile([P, SLOTS * WPAD], FP16, name=f"g{i}") for i in (0, 1)]
        fstage = pool.tile([P, PX], F32)
        rec32 = pool.tile([P, PX], F32)     # also reused as out32
        ab16 = pool.tile([P, PX], FP16)     # abs scratch, later rec16h
        se16 = pool.tile([P, PX], FP16)     # asum partial (even ch)
        so16 = pool.tile([P, PX], FP16)     # asum partial (odd ch)
        sse = pool.tile([P, PX], FP16)      # ssum partial (even ch)
        sso = pool.tile([P, PX], FP16)      # ssum partial (odd ch)

        g3 = [t[:, :].rearrange("p (s w) -> p s w", w=WPAD) for t in gb]
        w4 = w9[:, :].rearrange("p (s r w) -> p s r w", s=9, r=4, w=W)
        pr4 = pr[:, :].rearrange("p (s r w) -> p s r w", s=9, r=4, w=W)
        pr2 = pr[:, :].rearrange("p (s x) -> p s x", x=PX)

        def win(gv, col):
            # overlapping-window view [P, dr(3), r(4), w(640)] at col offset
            v = gv[:, 0:3, col:col + W]
            pstride = v.ap.to_list()[0][0]
            vv = v.copy()
            vv.ap = bass_rust.VecI64Pair(
                [[pstride, P], [WPAD, 3], [WPAD, 4], [1, W]])
            return vv

        for t in gb:
            nc.vector.memset(t[:, :], 0.0)

        # ---- feature load -> g0 (fp16) + halos ----
        feat_v = feat_d[:, :].rearrange("(p r) w -> p r w", r=R)
        nc.sync.dma_start(
            fstage[:, :].rearrange("p (r w) -> p r w", w=W), feat_v)
        nc.vector.tensor_copy(g3[0][:, 1:1 + R, 1:1 + W],
                              fstage[:, :].rearrange("p (r w) -> p r w", w=W))
        nc.sync.dma_start(g3[0][1:P, 0, :], g3[0][0:P - 1, R, :])
        nc.scalar.dma_start(g3[0][0:P - 1, SLOTS - 1, :], g3[0][1:P, 1, :])

        # ---- affinity load + weight prep (overlapped with DMA) ----
        aff_v = aff_d[:, :].rearrange("c (p x) -> c p x", x=PX)
        for c in range(CH):
            st = pool.tile([P, PX], F32, name=f"stage{c}", tag="stg", bufs=2)
            (nc.sync if c % 2 == 0 else nc.scalar).dma_start(
                st[:, :], aff_v[c])
            slab = w9[:, CH2SLAB[c] * PX:(CH2SLAB[c] + 1) * PX]
            # fp16 copy of channel into its slab (x0.5 folded into rec16h)
            nc.scalar.activation(slab, st[:, :], AF.Copy)
            absdst = se16 if c == 0 else (so16 if c == 1 else ab16)
            nc.scalar.activation(absdst[:, :], st[:, :], AF.Abs)
            if c == 0:
                nc.vector.tensor_copy(sse[:, :], slab)
            elif c == 1:
                nc.vector.tensor_copy(sso[:, :], slab)
            else:
                acc = se16 if c % 2 == 0 else so16
                sacc = sse if c % 2 == 0 else sso
                nc.vector.tensor_add(acc[:, :], acc[:, :], absdst[:, :])
                nc.vector.tensor_add(sacc[:, :], sacc[:, :], slab)
        nc.vector.tensor_add(se16[:, :], se16[:, :], so16[:, :])   # asum
        nc.vector.tensor_add(sse[:, :], sse[:, :], sso[:, :])      # ssum
        nc.vector.tensor_copy(fstage[:, :], se16[:, :])            # -> fp32
        scr = pool.tile([P, PX], F32, name="rscr", tag="stg", bufs=2)
        nc.vector.reciprocal_approx_accurate(rec32[:, :], fstage[:, :],
                                             scr[:, :])
        # rec16h = fp16(rec/2): folds the per-iteration 1/2 into weights
        nc.vector.tensor_scalar_mul(ab16[:, :], rec32[:, :], 0.5)
        # normalize the 8 channel slabs (in-place, rec16h broadcast)
        rb7 = ab16[:, :].unsqueeze(1).broadcast_to([P, 7, PX])
        w7a = w9[:, 0:7 * PX].rearrange("p (s x) -> p s x", x=PX)
        w7b = w9[:, 0:7 * PX].rearrange("p (s x) -> p s x", x=PX)
        nc.vector.tensor_mul(w7a, w7b, rb7)
        s8a = w9[:, 8 * PX:9 * PX]
        s8b = w9[:, 8 * PX:9 * PX]
        nc.vector.tensor_mul(s8a, s8b, ab16[:, :])
        # center slab: 0.5 - ssum*rec16h
        nc.vector.scalar_tensor_tensor(so16[:, :], sse[:, :], -1.0,
                                       ab16[:, :], OP.mult, OP.mult)
        nc.vector.tensor_scalar_add(w9[:, CEN_SLAB * PX:(CEN_SLAB + 1) * PX],
                                    so16[:, :], 0.5)

        # ---- iterations ----
        for it in range(times):
            cur = g3[it % 2]
            nxt = g3[(it + 1) % 2]
            # 9 tap products, 3 batched muls (dc = -1, +1, 0)
            nc.vector.tensor_mul(pr4[:, 0:3], win(cur, 0), w4[:, 0:3])
            nc.vector.tensor_mul(pr4[:, 3:6], win(cur, 2), w4[:, 3:6])
            nc.vector.tensor_mul(pr4[:, 6:9], win(cur, 1), w4[:, 6:9])
            # add tree
            nc.vector.tensor_add(pr2[:, 0:4], pr2[:, 0:4], pr2[:, 4:8])
            nc.vector.tensor_add(pr2[:, 0:2], pr2[:, 0:2], pr2[:, 2:4])
            nc.vector.tensor_add(pr2[:, 0], pr2[:, 0], pr2[:, 1])
            # final add: boundary rows first so halo DMAs overlap interior
            nc.vector.tensor_add(nxt[:, 1:5:3, 1:1 + W],
                                 pr4[:, 0, 0:4:3, :], pr4[:, 8, 0:4:3, :])
            if it != times - 1:
                nc.sync.dma_start(nxt[1:P, 0, :], nxt[0:P - 1, R, :])
                nc.scalar.dma_start(nxt[0:P - 1, SLOTS - 1, :],
                                    nxt[1:P, 1, :])
            nc.vector.tensor_add(nxt[:, 2:4, 1:1 + W],
                                 pr4[:, 0, 1:3, :], pr4[:, 8, 1:3, :])

        # ---- output: upscale by 2^times, store ----
        fin = g3[times % 2]
        out_v = out_d[:, :].rearrange("(p r) w -> p r w", r=R)
        o3 = rec32[:, :].rearrange("p (r w) -> p r w", w=W)
        scale = float(2.0 ** times)
        if times == 0:
            nc.vector.tensor_copy(o3, fstage[:, :].rearrange(
                "p (r w) -> p r w", w=W))
            nc.sync.dma_start(out_v, o3)
        else:
            nc.vector.tensor_scalar_mul(o3[:, 0:2, :], fin[:, 1:3, 1:1 + W],
                                        scale)
            nc.sync.dma_start(out_v[:, 0:2, :], o3[:, 0:2, :])
            nc.vector.tensor_scalar_mul(o3[:, 2:4, :], fin[:, 3:5, 1:1 + W],
                                        scale)
            nc.scalar.dma_start(out_v[:, 2:4, :], o3[:, 2:4, :])

    nc.compile()
    nc.m = get_hw_module(nc.m)
    return nc


_CACHE = {}


def _get(times: int):
    if times not in _CACHE:
        _CACHE[times] = _build(times)
    return _CACHE[times]


def kernel(affinity, feature, times, _trace=False, _trace_kwargs=None):
    t = int(times)
    nc = _get(t)
    aff = np.ascontiguousarray(affinity, dtype=np.float32)
    fea = np.ascontiguousarray(feature, dtype=np.float32)
    in_maps = [
        {"affinity": aff[b].reshape(CH, H * W), "feature": fea[b, 0]}
        for b in range(B)
    ]
    res = bass_utils.run_bass_kernel_spmd(
        nc, in_maps, core_ids=list(range(B)),
        trace=_trace, **(_trace_kwargs or {}),
    )
    out = np.stack([res.results[b]["out"] for b in range(B)])[:, None]
    if _trace:
        return out.astype(np.float32), res
    return out.astype(np.float32)


# revision 5
# speedup vs baseline: 1.1951x; 1.1951x over previous
"""AffinityPropagate prediction kernel for Trainium2 (8 NeuronCores).

Data-parallel over batch B=8: each core owns one image [480, 640].

Layout per core: 120 partitions x 4 image rows, rows padded to 642 cols.
State g is fp16 double-buffered with 2 halo row slots per partition,
g_k = f_k * 2^-k: the per-iteration 1/2 rescale is folded into the
weights (w' = w/2, exact), so each iteration is pure fp16 at DVE 2x:

  9 tap products as 6 batched muls (overlapping-window 4-dim APs, taps
  grouped by dc, rows split in half) + an 8-instruction half-split add
  tree. The two halves form independent chains so the scheduler hides
  the DVE write-ack latency between dependent ops. Halo rows refresh
  via 2 SBUF->SBUF DMAs overlapped with the other half's final add.
  Weight prep (normalize via reciprocal, center, *0.5) overlaps the
  affinity DMA, work split between ACT and DVE. Output is a 2x-mode
  tensor_scalar upscale by 2^times then DMA, split in halves.
"""

import numpy as np
from contextlib import ExitStack

import bass_rust
import concourse.bacc as bacc
import concourse.mybir as mybir
import concourse.tile as tile
from concourse import bass_utils
from concourse.bass_interp import get_hw_module

B, CH, H, W = 8, 8, 480, 640
P = 120             # partitions (each holds R rows)
R = H // P          # 4
SLOTS = R + 2       # + top/bottom halo row slots
WPAD = W + 2        # [0, img cols at 1..640, 0]
PX = R * W          # 2560 compact px per partition
HX = PX // 2        # half-image columns per partition

F32 = mybir.dt.float32
FP16 = mybir.dt.float16
AF = mybir.ActivationFunctionType
OP = mybir.AluOpType

# w9/pr slab order: 0..2 = dc=-1 (dr=-1,0,1), 3..5 = dc=+1, 6..8 = dc=0
# (dr=-1, center, dr=+1).  Affinity channel -> slab:
CH2SLAB = {0: 0, 3: 1, 5: 2, 2: 3, 4: 4, 7: 5, 1: 6, 6: 8}
CEN_SLAB = 7


def _build(times: int):
    nc = bacc.Bacc("TRN2", debug=False, dynamic_dma_scratch_size=2048)
    aff_d = nc.dram_tensor("affinity", [CH, H * W], F32, kind="ExternalInput")
    feat_d = nc.dram_tensor("feature", [H, W], F32, kind="ExternalInput")
    out_d = nc.dram_tensor("out", [H, W], F32, kind="ExternalOutput")

    with tile.TileContext(nc) as tc, ExitStack() as ctx:
        pool = ctx.enter_context(tc.tile_pool(name="main", bufs=1))

        w9 = pool.tile([P, 9 * PX], FP16)
        pr = pool.tile([P, 9 * PX], FP16)
        gb = [pool.tile([P, SLOTS * WPAD], FP16, name=f"g{i}") for i in (0, 1)]
        fstage = pool.tile([P, PX], F32)
        rec32 = pool.tile([P, PX], F32)     # also reused as out32
        ab16 = pool.tile([P, PX], FP16)     # abs scratch, later rec16h
        se16 = pool.tile([P, PX], FP16)     # asum partial (even ch)
        so16 = pool.tile([P, PX], FP16)     # asum partial (odd ch)
        sse = pool.tile([P, PX], FP16)      # ssum partial (even ch)
        sso = pool.tile([P, PX], FP16)      # ssum partial (odd ch)

        g3 = [t[:, :].rearrange("p (s w) -> p s w", w=WPAD) for t in gb]
        w4 = w9[:, :].rearrange("p (s r w) -> p s r w", s=9, r=4, w=W)
        pr4 = pr[:, :].rearrange("p (s r w) -> p s r w", s=9, r=4, w=W)
        pr2 = pr[:, :].rearrange("p (s x) -> p s x", x=PX)

        def win(gv, col, h):
            # overlapping-window view [P, dr(3), r(2), w(640)]: rows of
            # half h read slots h*2 + dr + r at column offset col
            v = gv[:, 2 * h:2 * h + 3, col:col + W]
            pstride = v.ap.to_list()[0][0]
            vv = v.copy()
            vv.ap = bass_rust.VecI64Pair(
                [[pstride, P], [WPAD, 3], [WPAD, 2], [1, W]])
            return vv

        for t in gb:
            nc.vector.memset(t[:, :], 0.0)

        # ---- feature load -> g0 (fp16) + halos ----
        feat_v = feat_d[:, :].rearrange("(p r) w -> p r w", r=R)
        nc.gpsimd.dma_start(
            fstage[:, :].rearrange("p (r w) -> p r w", w=W), feat_v)
        nc.vector.tensor_copy(g3[0][:, 1:1 + R, 1:1 + W],
                              fstage[:, :].rearrange("p (r w) -> p r w", w=W))
        nc.sync.dma_start(g3[0][1:P, 0, :], g3[0][0:P - 1, R, :])
        nc.scalar.dma_start(g3[0][0:P - 1, SLOTS - 1, :], g3[0][1:P, 1, :])

        # ---- affinity load + weight prep (overlapped with DMA) ----
        aff_v = aff_d[:, :].rearrange("c (p x) -> c p x", x=PX)
        for c in range(CH):
            st = pool.tile([P, PX], F32, name=f"stage{c}", tag="stg", bufs=2)
            (nc.sync if c % 2 == 0 else nc.scalar).dma_start(
                st[:, :], aff_v[c])
            slab = w9[:, CH2SLAB[c] * PX:(CH2SLAB[c] + 1) * PX]
            # fp16 copy of channel into its slab: split ACT/DVE to balance
            if c % 2 == 0:
                nc.scalar.activation(slab, st[:, :], AF.Copy)
            else:
                nc.vector.tensor_copy(slab, st[:, :])
            absdst = se16 if c == 0 else (so16 if c == 1 else ab16)
            nc.scalar.activation(absdst[:, :], st[:, :], AF.Abs)
            if c == 0:
                nc.vector.tensor_copy(sse[:, :], slab)
            elif c == 1:
                nc.vector.tensor_copy(sso[:, :], slab)
            else:
                acc = se16 if c % 2 == 0 else so16
                sacc = sse if c % 2 == 0 else sso
                nc.vector.tensor_add(acc[:, :], acc[:, :], absdst[:, :])
                nc.vector.tensor_add(sacc[:, :], sacc[:, :], slab)
        nc.vector.tensor_add(se16[:, :], se16[:, :], so16[:, :])   # asum
        nc.vector.tensor_add(sse[:, :], sse[:, :], sso[:, :])      # ssum
        nc.vector.tensor_copy(fstage[:, :], se16[:, :])            # -> fp32
        scr = pool.tile([P, PX], F32, name="rscr", tag="stg", bufs=2)
        nc.vector.reciprocal_approx_accurate(rec32[:, :], fstage[:, :],
                                             scr[:, :])
        # rec16h = fp16(rec/2): folds the per-iteration 1/2 into weights
        nc.vector.tensor_scalar_mul(ab16[:, :], rec32[:, :], 0.5)
        # normalize the 8 channel slabs in-place
        for k in list(range(7)) + [8]:
            sa = w9[:, k * PX:(k + 1) * PX]
            sb = w9[:, k * PX:(k + 1) * PX]
            nc.vector.tensor_mul(sa, sb, ab16[:, :])
        # center slab: 0.5 - ssum*rec16h
        nc.vector.scalar_tensor_tensor(so16[:, :], sse[:, :], -1.0,
                                       ab16[:, :], OP.mult, OP.mult)
        nc.vector.tensor_scalar_add(w9[:, CEN_SLAB * PX:(CEN_SLAB + 1) * PX],
                                    so16[:, :], 0.5)

        # ---- iterations: two independent half-chains (rows 0-1 / 2-3) ----
        for it in range(times):
            cur = g3[it % 2]
            nxt = g3[(it + 1) % 2]
            for h in (0, 1):
                nc.vector.tensor_mul(pr4[:, 0:3, 2 * h:2 * h + 2, :],
                                     win(cur, 0, h), w4[:, 0:3, 2 * h:2 * h + 2, :])
                nc.vector.tensor_mul(pr4[:, 3:6, 2 * h:2 * h + 2, :],
                                     win(cur, 2, h), w4[:, 3:6, 2 * h:2 * h + 2, :])
                nc.vector.tensor_mul(pr4[:, 6:9, 2 * h:2 * h + 2, :],
                                     win(cur, 1, h), w4[:, 6:9, 2 * h:2 * h + 2, :])
            for h in (0, 1):
                c0, c1 = h * HX, (h + 1) * HX
                nc.vector.tensor_add(pr2[:, 0:4, c0:c1], pr2[:, 0:4, c0:c1],
                                     pr2[:, 4:8, c0:c1])
            for h in (0, 1):
                c0, c1 = h * HX, (h + 1) * HX
                nc.vector.tensor_add(pr2[:, 0:2, c0:c1], pr2[:, 0:2, c0:c1],
                                     pr2[:, 2:4, c0:c1])
            for h in (0, 1):
                c0, c1 = h * HX, (h + 1) * HX
                nc.vector.tensor_add(pr2[:, 0, c0:c1], pr2[:, 0, c0:c1],
                                     pr2[:, 1, c0:c1])
            for h in (0, 1):
                nc.vector.tensor_add(nxt[:, 1 + 2 * h:3 + 2 * h, 1:1 + W],
                                     pr4[:, 0, 2 * h:2 * h + 2, :],
                                     pr4[:, 8, 2 * h:2 * h + 2, :])
                if it != times - 1:
                    if h == 0:   # bottom halo needs row 0 (half 0)
                        nc.sync.dma_start(nxt[0:P - 1, SLOTS - 1, :],
                                          nxt[1:P, 1, :])
                    else:        # top halo needs row 3 (half 1)
                        nc.scalar.dma_start(nxt[1:P, 0, :],
                                            nxt[0:P - 1, R, :])

        # ---- output: upscale by 2^times, store ----
        fin = g3[times % 2]
        out_v = out_d[:, :].rearrange("(p r) w -> p r w", r=R)
        o3 = rec32[:, :].rearrange("p (r w) -> p r w", w=W)
        scale = float(2.0 ** times)
        if times == 0:
            nc.vector.tensor_copy(o3, fstage[:, :].rearrange(
                "p (r w) -> p r w", w=W))
            nc.sync.dma_start(out_v, o3)
        else:
            nc.vector.tensor_scalar_mul(o3[:, 0:2, :], fin[:, 1:3, 1:1 + W],
                                        scale)
            nc.sync.dma_start(out_v[:, 0:2, :], o3[:, 0:2, :])
            nc.vector.tensor_scalar_mul(o3[:, 2:4, :], fin[:, 3:5, 1:1 + W],
                                        scale)
            nc.scalar.dma_start(out_v[:, 2:4, :], o3[:, 2:4, :])

    nc.compile()
    nc.m = get_hw_module(nc.m)
    return nc


_CACHE = {}


def _get(times: int):
    if times not in _CACHE:
        _CACHE[times] = _build(times)
    return _CACHE[times]


def kernel(affinity, feature, times, _trace=False, _trace_kwargs=None):
    t = int(times)
    nc = _get(t)
    aff = np.ascontiguousarray(affinity, dtype=np.float32)
    fea = np.ascontiguousarray(feature, dtype=np.float32)
    in_maps = [
        {"affinity": aff[b].reshape(CH, H * W), "feature": fea[b, 0]}
        for b in range(B)
    ]
    res = bass_utils.run_bass_kernel_spmd(
        nc, in_maps, core_ids=list(range(B)),
        trace=_trace, **(_trace_kwargs or {}),
    )
    out = np.stack([res.results[b]["out"] for b in range(B)])[:, None]
    if _trace:
        return out.astype(np.float32), res
    return out.astype(np.float32)
